# revision 17
# baseline (speedup 1.0000x reference)
"""Trainium2 Bass kernel for causal self-attention (B=4, T=2048, C=2048, H=16).

Sharding: 16 heads across 8 cores (2 heads/core), tensor-parallel column split
of Wqkv and row split of Wout; host sums the 8 row-parallel partial outputs.

Device-side layout strategy (everything "transposed", token index in the free
dimension), which makes every matmul land in its natural layout with zero
on-chip transposes:
  qT/kT   [D=128 part, T free]   = Wq_h^T @ x  (m-tile of the qkv projection)
  V       [T part, D free]       computed with x-blocks as the stationary operand
  S^T     [k part, q free]       = kT-block^T-contraction (lhsT=kT blk, rhs=qT)
  U = exp(S^T), causal blocks skipped entirely, diagonal blocks masked via a
      triangular 0/1 multiply after exp
  y^T     [D part, q free]       = sum_kb V_blk.T @ U_blk  (PSUM accumulate)
  denom   [1, q]                 = ones.T @ U_blk          (PSUM accumulate)
  y_norm  = y^T * broadcast(1/denom)  (K=1 fp32 matmul broadcasts the recip row)
  partial [tokens part, Cout]    lhsT = y^T block, rhs = Wout rows

RoPE is applied in [D, T] layout: the q/k columns of Wqkv are permuted on the
host so rotation pairs land at partitions (i, i+64) ("deinterleaved"); the
half-swap is two 64-partition DVE copies, then 3 elementwise ops against
host-precomputed [128, T] cos/sin tables (the 1/sqrt(D) scale is folded into
the q tables). Scores are invariant to any fixed permutation of head dims
applied to both q and k, so the reference is reproduced exactly.
"""

import math

import numpy as np
import ml_dtypes

# Module-level knobs (test harness may set TRACE=True to capture a profile).
TRACE = False
LAST_RESULT = None  # BassKernelResults of the last run (for profiling)

_B, _T, _C, _H = 4, 2048, 2048, 16
_D = 128
_CH = 512  # free-dim chunk (one PSUM bank of fp32)


def build_program(B, T, C, COUT, HPC, n_cores=8):
    """Build the per-core Bass/Tile program (identical on all cores)."""
    import concourse.bass as bass  # noqa: F401
    import concourse.mybir as mybir
    import concourse.tile as tile
    from concourse import bacc
    from contextlib import ExitStack

    dt = mybir.dt
    f32, bf16 = dt.float32, dt.bfloat16
    D, CH = _D, _CH
    KB = C // 128          # contraction blocks for the projections
    NCH = T // CH          # 512-token chunks per batch (also q-chunks)
    NTT = T // 128         # 128-token tiles per batch (also k-tiles)
    BT = B * T
    F = HPC * D            # per-core head feature width
    AF = mybir.ActivationFunctionType

    nc = bacc.Bacc(
        "TRN2", target_bir_lowering=False, debug=False, num_devices=n_cores
    )

    xt = nc.dram_tensor("xt", [C, BT], bf16, kind="ExternalInput").ap()
    wq = nc.dram_tensor("wq", [C, F], bf16, kind="ExternalInput").ap()
    wk = nc.dram_tensor("wk", [C, F], bf16, kind="ExternalInput").ap()
    wv = nc.dram_tensor("wv", [C, F], bf16, kind="ExternalInput").ap()
    wo = nc.dram_tensor("wo", [F, COUT], bf16, kind="ExternalInput").ap()
    ccq = nc.dram_tensor("ccq", [128, T], bf16, kind="ExternalInput").ap()
    ssq = nc.dram_tensor("ssq", [128, T], bf16, kind="ExternalInput").ap()
    cck = nc.dram_tensor("cck", [128, T], bf16, kind="ExternalInput").ap()
    ssk = nc.dram_tensor("ssk", [128, T], bf16, kind="ExternalInput").ap()
    triu = nc.dram_tensor("triu", [128, 128], bf16, kind="ExternalInput").ap()
    onesb = nc.dram_tensor("onesb", [128, 1], bf16, kind="ExternalInput").ap()
    part = nc.dram_tensor("part", [BT, COUT], bf16, kind="ExternalOutput").ap()

    with ExitStack() as ctx:
        tc = ctx.enter_context(tile.TileContext(nc))
        const_pool = ctx.enter_context(tc.tile_pool(name="const", bufs=1))
        xc_pool = ctx.enter_context(tc.tile_pool(name="xc", bufs=2))
        rot_pool = ctx.enter_context(tc.tile_pool(name="rot", bufs=4 * HPC))
        swap_pool = ctx.enter_context(tc.tile_pool(name="swap", bufs=3))
        a_pool = ctx.enter_context(tc.tile_pool(name="ropeA", bufs=3))
        v_pool = ctx.enter_context(tc.tile_pool(name="vsb", bufs=2))
        u_pool = ctx.enter_context(tc.tile_pool(name="u", bufs=6))
        r_pool = ctx.enter_context(tc.tile_pool(name="recip", bufs=2))
        rb_pool = ctx.enter_context(tc.tile_pool(name="rbcast", bufs=2))
        y_pool = ctx.enter_context(tc.tile_pool(name="y", bufs=2 * HPC))
        o_pool = ctx.enter_context(tc.tile_pool(name="osb", bufs=3))
        ps_qk = ctx.enter_context(tc.tile_pool(name="psqk", bufs=2, space="PSUM"))
        ps_v = ctx.enter_context(tc.tile_pool(name="psv", bufs=1, space="PSUM"))
        ps_S = ctx.enter_context(tc.tile_pool(name="psS", bufs=2, space="PSUM"))
        ps_y = ctx.enter_context(tc.tile_pool(name="psy", bufs=2, space="PSUM"))
        ps_d = ctx.enter_context(tc.tile_pool(name="psd", bufs=1, space="PSUM"))
        ps_R = ps_qk  # broadcast tiles share the projection PSUM slots

        # ---- persistent constants ----
        # load order matters for startup latency: q/k weights + rope tables
        # feed the first matmuls; wv is needed a bit later, wo only at the
        # first output projection.
        wq_sb = const_pool.tile([128, KB * F], bf16, tag="wq")
        wk_sb = const_pool.tile([128, KB * F], bf16, tag="wk")
        for wsb, wdr in ((wq_sb, wq), (wk_sb, wk)):
            nc.gpsimd.dma_start(
                wsb[:].rearrange("p (kb f) -> p kb f", kb=KB),
                wdr.rearrange("(kb p) f -> p kb f", p=128),
            )
        ccq_sb = const_pool.tile([128, T], bf16, tag="ccq")
        ssq_sb = const_pool.tile([128, T], bf16, tag="ssq")
        cck_sb = const_pool.tile([128, T], bf16, tag="cck")
        ssk_sb = const_pool.tile([128, T], bf16, tag="ssk")
        for csb, cdr in ((ccq_sb, ccq), (ssq_sb, ssq), (cck_sb, cck), (ssk_sb, ssk)):
            nc.gpsimd.dma_start(csb[:], cdr)
        wv_sb = const_pool.tile([128, KB * F], bf16, tag="wv")
        nc.gpsimd.dma_start(
            wv_sb[:].rearrange("p (kb f) -> p kb f", kb=KB),
            wv.rearrange("(kb p) f -> p kb f", p=128),
        )
        triu_sb = const_pool.tile([128, 128], bf16, tag="triu")
        nc.gpsimd.dma_start(triu_sb[:], triu)
        onesb_sb = const_pool.tile([128, 1], bf16, tag="onesb")
        nc.gpsimd.dma_start(onesb_sb[:], onesb)
        wo_sb = const_pool.tile([128, HPC * COUT], bf16, tag="wo")
        nc.gpsimd.dma_start(
            wo_sb[:].rearrange("p (h n) -> p h n", h=HPC),
            wo.rearrange("(h p) n -> p h n", p=128),
        )

        # ---- PE warm-up: keep the HAM clock gate busy while the initial
        # DMA wave streams in, so the first real matmuls run at 2.4 GHz ----
        junk = const_pool.tile([128, 128], bf16, tag="warmjunk")
        nc.vector.memset(junk[:], 0)
        psw = ps_S.tile([128, 128], f32, tag="psS", name="pswarm")
        for _ in range(220):
            nc.tensor.matmul(
                psw[:], junk[:], junk[:], start=True, stop=True,
                skip_group_check=True,
            )

        # Deferred out-projection units (one q-chunk of token tiles each).
        # Emitted with a one-chunk lag so the softmax-normalization chain of
        # chunk j completes while PE runs chunk j+1's attention matmuls —
        # emitting immediately would park that latency on PE's in-order path.
        pending = []

        def emit_outproj():
            ysb_, b_, j_ = pending.pop(0)
            for m in range(j_ * (CH // 128), (j_ + 1) * (CH // 128)):
                osb = o_pool.tile([128, COUT], bf16, tag="osb", name="osb")
                for nch in range(COUT // CH):
                    pso = ps_qk.tile([128, CH], f32, tag="psqk", name="pso")
                    for h in range(HPC):
                        nc.tensor.matmul(
                            pso[:],
                            ysb_[h][:, m * 128 : (m + 1) * 128],
                            wo_sb[
                                :,
                                h * COUT + nch * CH : h * COUT + (nch + 1) * CH,
                            ],
                            start=(h == 0),
                            stop=(h == HPC - 1),
                            skip_group_check=True,
                        )
                    if nch % 2 == 0:
                        nc.scalar.copy(osb[:, nch * CH : (nch + 1) * CH], pso[:])
                    else:
                        nc.vector.tensor_copy(
                            osb[:, nch * CH : (nch + 1) * CH], pso[:]
                        )
                nc.sync.dma_start(
                    part[b_ * T + m * 128 : b_ * T + (m + 1) * 128, :], osb[:]
                )

        for b in range(B):
            # ---- qkv projection + RoPE for this batch ----
            qrot = [rot_pool.tile([128, T], bf16, tag="rot", name=f"qrot{h}") for h in range(HPC)]
            krot = [rot_pool.tile([128, T], bf16, tag="rot", name=f"krot{h}") for h in range(HPC)]
            vsb = v_pool.tile([128, NTT * F], bf16, tag="v")
            for c in range(NCH):
                xc = xc_pool.tile([128, KB * CH], bf16, tag="xc")
                nc.sync.dma_start(
                    xc[:].rearrange("p (kb t) -> p kb t", kb=KB),
                    xt.rearrange("(kb p) t -> p kb t", p=128)[
                        :, :, b * T + c * CH : b * T + (c + 1) * CH
                    ],
                )
                for h in range(HPC):
                    for wsb, ccs, sss, dst in (
                        (wq_sb, ccq_sb, ssq_sb, qrot[h]),
                        (wk_sb, cck_sb, ssk_sb, krot[h]),
                    ):
                        ps = ps_qk.tile([128, CH], f32, tag="psqk")
                        for kb in range(KB):
                            nc.tensor.matmul(
                                ps[:],
                                wsb[:, kb * F + h * D : kb * F + (h + 1) * D],
                                xc[:, kb * CH : (kb + 1) * CH],
                                start=(kb == 0),
                                stop=(kb == KB - 1),
                            )
                        # RoPE: rot = ps * cc + halfswap(ps) * ss
                        sw = swap_pool.tile([128, CH], bf16, tag="swap")
                        nc.vector.tensor_copy(sw[0:64, :], ps[64:128, :])
                        nc.vector.tensor_copy(sw[64:128, :], ps[0:64, :])
                        A = a_pool.tile([128, CH], f32, tag="ropeA")
                        nc.vector.tensor_mul(
                            A[:], ps[:], ccs[:, c * CH : (c + 1) * CH]
                        )
                        Bt = a_pool.tile([128, CH], bf16, tag="ropeB")
                        nc.vector.tensor_mul(
                            Bt[:], sw[:], sss[:, c * CH : (c + 1) * CH]
                        )
                        nc.vector.tensor_add(
                            dst[:, c * CH : (c + 1) * CH], A[:], Bt[:]
                        )
                # V in [token part, feature free] layout: x-blocks stationary
                for tm in range(CH // 128):
                    psv = ps_v.tile([128, F], f32, tag="psv")
                    for kb in range(KB):
                        nc.tensor.matmul(
                            psv[:],
                            xc[:, kb * CH + tm * 128 : kb * CH + tm * 128 + 128],
                            wv_sb[:, kb * F : (kb + 1) * F],
                            start=(kb == 0),
                            stop=(kb == KB - 1),
                        )
                    tt = c * (CH // 128) + tm
                    nc.scalar.copy(vsb[:, tt * F : (tt + 1) * F], psv[:])

            # ---- attention per head, out-projection interleaved per q-chunk ----
            ysb = [y_pool.tile([128, T], bf16, tag="y", name=f"ysb{h}") for h in range(HPC)]
            for j in range(NCH):
                for h in range(HPC):
                    psy = ps_y.tile([128, CH], f32, tag="psy")
                    psd = ps_d.tile([1, CH], f32, tag="psd")
                    nkb = (CH // 128) * (j + 1)  # causal: k-tiles <= q-chunk end
                    for kb in range(nkb):
                        c0 = max(0, kb * 128 - j * CH)
                        psS = ps_S.tile([128, CH], f32, tag="psS")
                        nc.tensor.matmul(
                            psS[:, c0:CH],
                            krot[h][:, kb * 128 : (kb + 1) * 128],
                            qrot[h][:, j * CH + c0 : (j + 1) * CH],
                            start=True,
                            stop=True,
                        )
                        U = u_pool.tile([128, CH], bf16, tag="u")
                        nc.scalar.activation(U[:, c0:CH], psS[:, c0:CH], AF.Exp)
                        if kb * 128 >= j * CH:
                            # diagonal 128x128 block: zero out k > q after exp
                            nc.vector.tensor_mul(
                                U[:, c0 : c0 + 128],
                                U[:, c0 : c0 + 128],
                                triu_sb[:],
                            )
                        nc.tensor.matmul(
                            psy[:, c0:CH],
                            vsb[:, kb * F + h * D : kb * F + (h + 1) * D],
                            U[:, c0:CH],
                            start=(kb == 0),
                            stop=(kb == nkb - 1),
                            skip_group_check=True,
                        )
                        nc.tensor.matmul(
                            psd[:, c0:CH],
                            onesb_sb[:],
                            U[:, c0:CH],
                            start=(kb == 0),
                            stop=(kb == nkb - 1),
                            skip_group_check=True,
                        )
                    rr = r_pool.tile([1, CH], f32, tag="recip")
                    nc.vector.reciprocal_approx_fast(rr[:], psd[:])
                    Rsb = rb_pool.tile([128, CH], f32, tag="rbcast")
                    nc.gpsimd.partition_broadcast(Rsb[:], rr[:])
                    nc.vector.tensor_mul(
                        ysb[h][:, j * CH : (j + 1) * CH], psy[:], Rsb[:]
                    )

                # defer this chunk's out-projection by one chunk
                pending.append((ysb, b, j))
                if len(pending) > 1:
                    emit_outproj()

        while pending:
            emit_outproj()

    nc.compile()
    return nc


def make_host_inputs(x, cos, sin, Wqkv, Wout, H, n_cores):
    """Shard + precompute the per-core device input maps (numpy, host side)."""
    bf16 = ml_dtypes.bfloat16
    B, T, C = x.shape
    D = C // H
    HPC = H // n_cores
    COUT = Wout.shape[1]

    xt = np.ascontiguousarray(x.reshape(B * T, C).T).astype(bf16)

    # deinterleave permutation within each head: [0,2,4,...,1,3,5,...]
    perm = np.concatenate([np.arange(0, D, 2), np.arange(1, D, 2)])
    Wq = Wqkv[:, 0:C].reshape(C, H, D)[:, :, perm]
    Wk = Wqkv[:, C : 2 * C].reshape(C, H, D)[:, :, perm]
    Wv = Wqkv[:, 2 * C : 3 * C].reshape(C, H, D)

    cosT = cos.T  # [D/2, T]
    CC = np.concatenate([cosT, cosT], axis=0)  # [D, T]
    SS = np.concatenate([-sin.T, sin.T], axis=0)
    scale = 1.0 / math.sqrt(D)
    ccq = (CC * scale).astype(bf16)
    ssq = (SS * scale).astype(bf16)
    cck = CC.astype(bf16)
    ssk = SS.astype(bf16)

    tri = np.triu(np.ones((128, 128), dtype=np.float32)).astype(bf16)
    onesb = np.ones((128, 1), dtype=np.float32).astype(bf16)

    in_maps = []
    for core in range(n_cores):
        hs = slice(core * HPC, (core + 1) * HPC)
        in_maps.append(
            {
                "xt": xt,
                "wq": np.ascontiguousarray(
                    Wq[:, hs, :].reshape(C, HPC * D)
                ).astype(bf16),
                "wk": np.ascontiguousarray(
                    Wk[:, hs, :].reshape(C, HPC * D)
                ).astype(bf16),
                "wv": np.ascontiguousarray(
                    Wv[:, hs, :].reshape(C, HPC * D)
                ).astype(bf16),
                "wo": np.ascontiguousarray(
                    Wout[core * HPC * D : (core + 1) * HPC * D, :]
                ).astype(bf16),
                "ccq": ccq,
                "ssq": ssq,
                "cck": cck,
                "ssk": ssk,
                "triu": tri,
                "onesb": onesb,
            }
        )
    return in_maps


_PROGRAM_CACHE = {}


def kernel(x, cos, sin, Wqkv, Wout):
    global LAST_RESULT
    from concourse.bass_utils import run_bass_kernel_spmd

    x = np.asarray(x, dtype=np.float32)
    cos = np.asarray(cos, dtype=np.float32)
    sin = np.asarray(sin, dtype=np.float32)
    Wqkv = np.asarray(Wqkv, dtype=np.float32)
    Wout = np.asarray(Wout, dtype=np.float32)

    B, T, C = x.shape
    H = _H
    COUT = Wout.shape[1]
    n_cores = 8
    HPC = H // n_cores

    key = (B, T, C, COUT, HPC, n_cores)
    if key not in _PROGRAM_CACHE:
        _PROGRAM_CACHE[key] = build_program(B, T, C, COUT, HPC, n_cores)
    nc = _PROGRAM_CACHE[key]

    in_maps = make_host_inputs(x, cos, sin, Wqkv, Wout, H, n_cores)
    res = run_bass_kernel_spmd(
        nc, in_maps, core_ids=list(range(n_cores)), trace=TRACE
    )
    LAST_RESULT = res

    out = np.zeros((B * T, COUT), dtype=np.float32)
    for r in res.results:
        out += np.asarray(r["part"], dtype=np.float32)
    return out.reshape(B, T, COUT)


# revision 18
# speedup vs baseline: 1.1924x; 1.1924x over previous
"""Trainium2 Bass kernel for causal self-attention (B=4, T=2048, C=2048, H=16).

Sharding: 16 heads across 8 cores (2 heads/core), tensor-parallel column split
of Wqkv and row split of Wout; host sums the 8 row-parallel partial outputs.

Device-side layout strategy (everything "transposed", token index in the free
dimension), which makes every matmul land in its natural layout with zero
on-chip transposes:
  qT/kT   [D=128 part, T free]   = Wq_h^T @ x  (m-tile of the qkv projection)
  V       [T part, D free]       computed with x-blocks as the stationary operand
  S^T     [k part, q free]       = kT-block^T-contraction (lhsT=kT blk, rhs=qT)
  U = exp(S^T), causal blocks skipped entirely, diagonal blocks masked via a
      triangular 0/1 multiply after exp
  y^T     [D part, q free]       = sum_kb V_blk.T @ U_blk  (PSUM accumulate)
  denom   [1, q]                 = ones.T @ U_blk          (PSUM accumulate)
  y_norm  = y^T * broadcast(1/denom)  (K=1 fp32 matmul broadcasts the recip row)
  partial [tokens part, Cout]    lhsT = y^T block, rhs = Wout rows

RoPE is applied in [D, T] layout: the q/k columns of Wqkv are permuted on the
host so rotation pairs land at partitions (i, i+64) ("deinterleaved"); the
half-swap is two 64-partition DVE copies, then 3 elementwise ops against
host-precomputed [128, T] cos/sin tables (the 1/sqrt(D) scale is folded into
the q tables). Scores are invariant to any fixed permutation of head dims
applied to both q and k, so the reference is reproduced exactly.
"""

import math

import numpy as np
import ml_dtypes

# Module-level knobs (test harness may set TRACE=True to capture a profile).
TRACE = False
LAST_RESULT = None  # BassKernelResults of the last run (for profiling)

_B, _T, _C, _H = 4, 2048, 2048, 16
_D = 128
_CH = 512  # free-dim chunk (one PSUM bank of fp32)


def build_program(B, T, C, COUT, HPC, n_cores=8):
    """Build the per-core Bass/Tile program (identical on all cores)."""
    import concourse.bass as bass  # noqa: F401
    import concourse.mybir as mybir
    import concourse.tile as tile
    from concourse import bacc
    from contextlib import ExitStack

    dt = mybir.dt
    f32, bf16 = dt.float32, dt.bfloat16
    D, CH = _D, _CH
    KB = C // 128          # contraction blocks for the projections
    NCH = T // CH          # 512-token chunks per batch (also q-chunks)
    NTT = T // 128         # 128-token tiles per batch (also k-tiles)
    BT = B * T
    F = HPC * D            # per-core head feature width
    AF = mybir.ActivationFunctionType

    nc = bacc.Bacc(
        "TRN2", target_bir_lowering=False, debug=False, num_devices=n_cores
    )

    xt = nc.dram_tensor("xt", [C, BT], bf16, kind="ExternalInput").ap()
    wq = nc.dram_tensor("wq", [C, F], bf16, kind="ExternalInput").ap()
    wk = nc.dram_tensor("wk", [C, F], bf16, kind="ExternalInput").ap()
    wv = nc.dram_tensor("wv", [C, F], bf16, kind="ExternalInput").ap()
    wo = nc.dram_tensor("wo", [F, COUT], bf16, kind="ExternalInput").ap()
    ccq = nc.dram_tensor("ccq", [128, T], bf16, kind="ExternalInput").ap()
    ssq = nc.dram_tensor("ssq", [128, T], bf16, kind="ExternalInput").ap()
    cck = nc.dram_tensor("cck", [128, T], bf16, kind="ExternalInput").ap()
    ssk = nc.dram_tensor("ssk", [128, T], bf16, kind="ExternalInput").ap()
    triu = nc.dram_tensor("triu", [128, 128], bf16, kind="ExternalInput").ap()
    onesb = nc.dram_tensor("onesb", [128, 1], bf16, kind="ExternalInput").ap()
    part = nc.dram_tensor("part", [BT, COUT], bf16, kind="ExternalOutput").ap()

    with ExitStack() as ctx:
        tc = ctx.enter_context(tile.TileContext(nc))
        const_pool = ctx.enter_context(tc.tile_pool(name="const", bufs=1))
        xc_pool = ctx.enter_context(tc.tile_pool(name="xc", bufs=2))
        rot_pool = ctx.enter_context(tc.tile_pool(name="rot", bufs=4 * HPC))
        swap_pool = ctx.enter_context(tc.tile_pool(name="swap", bufs=3))
        a_pool = ctx.enter_context(tc.tile_pool(name="ropeA", bufs=3))
        v_pool = ctx.enter_context(tc.tile_pool(name="vsb", bufs=2))
        u_pool = ctx.enter_context(tc.tile_pool(name="u", bufs=6))
        r_pool = ctx.enter_context(tc.tile_pool(name="recip", bufs=2))
        rb_pool = ctx.enter_context(tc.tile_pool(name="rbcast", bufs=2))
        y_pool = ctx.enter_context(tc.tile_pool(name="y", bufs=2 * HPC))
        o_pool = ctx.enter_context(tc.tile_pool(name="osb", bufs=3))
        ps_qk = ctx.enter_context(tc.tile_pool(name="psqk", bufs=2, space="PSUM"))
        ps_v = ctx.enter_context(tc.tile_pool(name="psv", bufs=1, space="PSUM"))
        ps_S = ctx.enter_context(tc.tile_pool(name="psS", bufs=2, space="PSUM"))
        ps_y = ctx.enter_context(tc.tile_pool(name="psy", bufs=2, space="PSUM"))
        ps_d = ctx.enter_context(tc.tile_pool(name="psd", bufs=1, space="PSUM"))
        ps_R = ps_qk  # broadcast tiles share the projection PSUM slots

        # ---- persistent constants ----
        # load order matters for startup latency: q/k weights + rope tables
        # feed the first matmuls; wv is needed a bit later, wo only at the
        # first output projection.
        wq_sb = const_pool.tile([128, KB * F], bf16, tag="wq")
        wk_sb = const_pool.tile([128, KB * F], bf16, tag="wk")
        for wsb, wdr in ((wq_sb, wq), (wk_sb, wk)):
            nc.sync.dma_start(
                wsb[:].rearrange("p (kb f) -> p kb f", kb=KB),
                wdr.rearrange("(kb p) f -> p kb f", p=128),
            )
        xc0 = xc_pool.tile([128, KB * CH], bf16, tag="xc", name="xc0")
        nc.sync.dma_start(
            xc0[:].rearrange("p (kb t) -> p kb t", kb=KB),
            xt.rearrange("(kb p) t -> p kb t", p=128)[:, :, 0:CH],
        )
        ccq_sb = const_pool.tile([128, T], bf16, tag="ccq")
        ssq_sb = const_pool.tile([128, T], bf16, tag="ssq")
        cck_sb = const_pool.tile([128, T], bf16, tag="cck")
        ssk_sb = const_pool.tile([128, T], bf16, tag="ssk")
        for csb, cdr in ((ccq_sb, ccq), (ssq_sb, ssq), (cck_sb, cck), (ssk_sb, ssk)):
            nc.sync.dma_start(csb[:], cdr)
        wv_sb = const_pool.tile([128, KB * F], bf16, tag="wv")
        nc.sync.dma_start(
            wv_sb[:].rearrange("p (kb f) -> p kb f", kb=KB),
            wv.rearrange("(kb p) f -> p kb f", p=128),
        )
        triu_sb = const_pool.tile([128, 128], bf16, tag="triu")
        nc.sync.dma_start(triu_sb[:], triu)
        onesb_sb = const_pool.tile([128, 1], bf16, tag="onesb")
        nc.sync.dma_start(onesb_sb[:], onesb)
        wo_sb = const_pool.tile([128, HPC * COUT], bf16, tag="wo")
        nc.sync.dma_start(
            wo_sb[:].rearrange("p (h n) -> p h n", h=HPC),
            wo.rearrange("(h p) n -> p h n", p=128),
        )

        # ---- PE warm-up: keep the HAM clock gate busy while the initial
        # DMA wave streams in, so the first real matmuls run at 2.4 GHz ----
        junk = const_pool.tile([128, 128], bf16, tag="warmjunk")
        nc.vector.memset(junk[:], 0)
        psw = ps_S.tile([128, 128], f32, tag="psS", name="pswarm")
        for _ in range(160):
            nc.tensor.matmul(
                psw[:], junk[:], junk[:], start=True, stop=True,
                skip_group_check=True,
            )

        # Deferred out-projection units (one q-chunk of token tiles each).
        # Emitted with a one-chunk lag so the softmax-normalization chain of
        # chunk j completes while PE runs chunk j+1's attention matmuls —
        # emitting immediately would park that latency on PE's in-order path.
        pending = []

        def emit_outproj():
            ysb_, b_, j_ = pending.pop(0)
            for m in range(j_ * (CH // 128), (j_ + 1) * (CH // 128)):
                osb = o_pool.tile([128, COUT], bf16, tag="osb", name="osb")
                for nch in range(COUT // CH):
                    pso = ps_qk.tile([128, CH], f32, tag="psqk", name="pso")
                    for h in range(HPC):
                        nc.tensor.matmul(
                            pso[:],
                            ysb_[h][:, m * 128 : (m + 1) * 128],
                            wo_sb[
                                :,
                                h * COUT + nch * CH : h * COUT + (nch + 1) * CH,
                            ],
                            start=(h == 0),
                            stop=(h == HPC - 1),
                            skip_group_check=True,
                        )
                    if nch % 2 == 0:
                        nc.scalar.copy(osb[:, nch * CH : (nch + 1) * CH], pso[:])
                    else:
                        nc.vector.tensor_copy(
                            osb[:, nch * CH : (nch + 1) * CH], pso[:]
                        )
                nc.sync.dma_start(
                    part[b_ * T + m * 128 : b_ * T + (m + 1) * 128, :], osb[:]
                )

        for b in range(B):
            # ---- qkv projection + RoPE for this batch ----
            qrot = [rot_pool.tile([128, T], bf16, tag="rot", name=f"qrot{h}") for h in range(HPC)]
            krot = [rot_pool.tile([128, T], bf16, tag="rot", name=f"krot{h}") for h in range(HPC)]
            vsb = v_pool.tile([128, NTT * F], bf16, tag="v")
            for c in range(NCH):
                if b == 0 and c == 0:
                    xc = xc0
                else:
                    xc = xc_pool.tile([128, KB * CH], bf16, tag="xc")
                    nc.sync.dma_start(
                        xc[:].rearrange("p (kb t) -> p kb t", kb=KB),
                        xt.rearrange("(kb p) t -> p kb t", p=128)[
                            :, :, b * T + c * CH : b * T + (c + 1) * CH
                        ],
                    )
                for h in range(HPC):
                    for wsb, ccs, sss, dst in (
                        (wq_sb, ccq_sb, ssq_sb, qrot[h]),
                        (wk_sb, cck_sb, ssk_sb, krot[h]),
                    ):
                        ps = ps_qk.tile([128, CH], f32, tag="psqk")
                        for kb in range(KB):
                            nc.tensor.matmul(
                                ps[:],
                                wsb[:, kb * F + h * D : kb * F + (h + 1) * D],
                                xc[:, kb * CH : (kb + 1) * CH],
                                start=(kb == 0),
                                stop=(kb == KB - 1),
                            )
                        # RoPE: rot = ps * cc + halfswap(ps) * ss
                        sw = swap_pool.tile([128, CH], bf16, tag="swap")
                        nc.vector.tensor_copy(sw[0:64, :], ps[64:128, :])
                        nc.vector.tensor_copy(sw[64:128, :], ps[0:64, :])
                        A = a_pool.tile([128, CH], f32, tag="ropeA")
                        nc.vector.tensor_mul(
                            A[:], ps[:], ccs[:, c * CH : (c + 1) * CH]
                        )
                        Bt = a_pool.tile([128, CH], bf16, tag="ropeB")
                        nc.vector.tensor_mul(
                            Bt[:], sw[:], sss[:, c * CH : (c + 1) * CH]
                        )
                        nc.vector.tensor_add(
                            dst[:, c * CH : (c + 1) * CH], A[:], Bt[:]
                        )
                # V in [token part, feature free] layout: x-blocks stationary
                for tm in range(CH // 128):
                    psv = ps_v.tile([128, F], f32, tag="psv")
                    for kb in range(KB):
                        nc.tensor.matmul(
                            psv[:],
                            xc[:, kb * CH + tm * 128 : kb * CH + tm * 128 + 128],
                            wv_sb[:, kb * F : (kb + 1) * F],
                            start=(kb == 0),
                            stop=(kb == KB - 1),
                        )
                    tt = c * (CH // 128) + tm
                    nc.scalar.copy(vsb[:, tt * F : (tt + 1) * F], psv[:])

            # ---- attention per head, out-projection interleaved per q-chunk ----
            ysb = [y_pool.tile([128, T], bf16, tag="y", name=f"ysb{h}") for h in range(HPC)]
            for j in range(NCH):
                for h in range(HPC):
                    psy = ps_y.tile([128, CH], f32, tag="psy")
                    psd = ps_d.tile([1, CH], f32, tag="psd")
                    nkb = (CH // 128) * (j + 1)  # causal: k-tiles <= q-chunk end
                    for kb in range(nkb):
                        c0 = max(0, kb * 128 - j * CH)
                        psS = ps_S.tile([128, CH], f32, tag="psS")
                        nc.tensor.matmul(
                            psS[:, c0:CH],
                            krot[h][:, kb * 128 : (kb + 1) * 128],
                            qrot[h][:, j * CH + c0 : (j + 1) * CH],
                            start=True,
                            stop=True,
                        )
                        U = u_pool.tile([128, CH], bf16, tag="u")
                        nc.scalar.activation(U[:, c0:CH], psS[:, c0:CH], AF.Exp)
                        if kb * 128 >= j * CH:
                            # diagonal 128x128 block: zero out k > q after exp
                            nc.vector.tensor_mul(
                                U[:, c0 : c0 + 128],
                                U[:, c0 : c0 + 128],
                                triu_sb[:],
                            )
                        nc.tensor.matmul(
                            psy[:, c0:CH],
                            vsb[:, kb * F + h * D : kb * F + (h + 1) * D],
                            U[:, c0:CH],
                            start=(kb == 0),
                            stop=(kb == nkb - 1),
                            skip_group_check=True,
                        )
                        nc.tensor.matmul(
                            psd[:, c0:CH],
                            onesb_sb[:],
                            U[:, c0:CH],
                            start=(kb == 0),
                            stop=(kb == nkb - 1),
                            skip_group_check=True,
                        )
                    rr = r_pool.tile([1, CH], f32, tag="recip")
                    nc.vector.reciprocal_approx_fast(rr[:], psd[:])
                    Rsb = rb_pool.tile([128, CH], f32, tag="rbcast")
                    nc.gpsimd.partition_broadcast(Rsb[:], rr[:])
                    nc.vector.tensor_mul(
                        ysb[h][:, j * CH : (j + 1) * CH], psy[:], Rsb[:]
                    )

                # defer this chunk's out-projection by one chunk
                pending.append((ysb, b, j))
                if len(pending) > 1:
                    emit_outproj()

        while pending:
            emit_outproj()

    nc.compile()
    return nc


def make_host_inputs(x, cos, sin, Wqkv, Wout, H, n_cores):
    """Shard + precompute the per-core device input maps (numpy, host side)."""
    bf16 = ml_dtypes.bfloat16
    B, T, C = x.shape
    D = C // H
    HPC = H // n_cores
    COUT = Wout.shape[1]

    xt = np.ascontiguousarray(x.reshape(B * T, C).T).astype(bf16)

    # deinterleave permutation within each head: [0,2,4,...,1,3,5,...]
    perm = np.concatenate([np.arange(0, D, 2), np.arange(1, D, 2)])
    Wq = Wqkv[:, 0:C].reshape(C, H, D)[:, :, perm]
    Wk = Wqkv[:, C : 2 * C].reshape(C, H, D)[:, :, perm]
    Wv = Wqkv[:, 2 * C : 3 * C].reshape(C, H, D)

    cosT = cos.T  # [D/2, T]
    CC = np.concatenate([cosT, cosT], axis=0)  # [D, T]
    SS = np.concatenate([-sin.T, sin.T], axis=0)
    scale = 1.0 / math.sqrt(D)
    ccq = (CC * scale).astype(bf16)
    ssq = (SS * scale).astype(bf16)
    cck = CC.astype(bf16)
    ssk = SS.astype(bf16)

    tri = np.triu(np.ones((128, 128), dtype=np.float32)).astype(bf16)
    onesb = np.ones((128, 1), dtype=np.float32).astype(bf16)

    in_maps = []
    for core in range(n_cores):
        hs = slice(core * HPC, (core + 1) * HPC)
        in_maps.append(
            {
                "xt": xt,
                "wq": np.ascontiguousarray(
                    Wq[:, hs, :].reshape(C, HPC * D)
                ).astype(bf16),
                "wk": np.ascontiguousarray(
                    Wk[:, hs, :].reshape(C, HPC * D)
                ).astype(bf16),
                "wv": np.ascontiguousarray(
                    Wv[:, hs, :].reshape(C, HPC * D)
                ).astype(bf16),
                "wo": np.ascontiguousarray(
                    Wout[core * HPC * D : (core + 1) * HPC * D, :]
                ).astype(bf16),
                "ccq": ccq,
                "ssq": ssq,
                "cck": cck,
                "ssk": ssk,
                "triu": tri,
                "onesb": onesb,
            }
        )
    return in_maps


_PROGRAM_CACHE = {}


def kernel(x, cos, sin, Wqkv, Wout):
    global LAST_RESULT
    from concourse.bass_utils import run_bass_kernel_spmd

    x = np.asarray(x, dtype=np.float32)
    cos = np.asarray(cos, dtype=np.float32)
    sin = np.asarray(sin, dtype=np.float32)
    Wqkv = np.asarray(Wqkv, dtype=np.float32)
    Wout = np.asarray(Wout, dtype=np.float32)

    B, T, C = x.shape
    H = _H
    COUT = Wout.shape[1]
    n_cores = 8
    HPC = H // n_cores

    key = (B, T, C, COUT, HPC, n_cores)
    if key not in _PROGRAM_CACHE:
        _PROGRAM_CACHE[key] = build_program(B, T, C, COUT, HPC, n_cores)
    nc = _PROGRAM_CACHE[key]

    in_maps = make_host_inputs(x, cos, sin, Wqkv, Wout, H, n_cores)
    res = run_bass_kernel_spmd(
        nc, in_maps, core_ids=list(range(n_cores)), trace=TRACE
    )
    LAST_RESULT = res

    out = np.zeros((B * T, COUT), dtype=np.float32)
    for r in res.results:
        out += np.asarray(r["part"], dtype=np.float32)
    return out.reshape(B, T, COUT)


# revision 19
# speedup vs baseline: 1.1948x; 1.0020x over previous
"""Trainium2 Bass kernel for causal self-attention (B=4, T=2048, C=2048, H=16).

Sharding: 16 heads across 8 cores (2 heads/core), tensor-parallel column split
of Wqkv and row split of Wout; host sums the 8 row-parallel partial outputs.

Device-side layout strategy (everything "transposed", token index in the free
dimension), which makes every matmul land in its natural layout with zero
on-chip transposes:
  qT/kT   [D=128 part, T free]   = Wq_h^T @ x  (m-tile of the qkv projection)
  V       [T part, D free]       computed with x-blocks as the stationary operand
  S^T     [k part, q free]       = kT-block^T-contraction (lhsT=kT blk, rhs=qT)
  U = exp(S^T), causal blocks skipped entirely, diagonal blocks masked via a
      triangular 0/1 multiply after exp
  y^T     [D part, q free]       = sum_kb V_blk.T @ U_blk  (PSUM accumulate)
  denom   [1, q]                 = ones.T @ U_blk          (PSUM accumulate)
  y_norm  = y^T * broadcast(1/denom)  (K=1 fp32 matmul broadcasts the recip row)
  partial [tokens part, Cout]    lhsT = y^T block, rhs = Wout rows

RoPE is applied in [D, T] layout: the q/k columns of Wqkv are permuted on the
host so rotation pairs land at partitions (i, i+64) ("deinterleaved"); the
half-swap is two 64-partition DVE copies, then 3 elementwise ops against
host-precomputed [128, T] cos/sin tables (the 1/sqrt(D) scale is folded into
the q tables). Scores are invariant to any fixed permutation of head dims
applied to both q and k, so the reference is reproduced exactly.
"""

import math

import numpy as np
import ml_dtypes

# Module-level knobs (test harness may set TRACE=True to capture a profile).
TRACE = False
LAST_RESULT = None  # BassKernelResults of the last run (for profiling)

_B, _T, _C, _H = 4, 2048, 2048, 16
_D = 128
_CH = 512  # free-dim chunk (one PSUM bank of fp32)


def build_program(B, T, C, COUT, HPC, n_cores=8):
    """Build the per-core Bass/Tile program (identical on all cores)."""
    import concourse.bass as bass  # noqa: F401
    import concourse.mybir as mybir
    import concourse.tile as tile
    from concourse import bacc
    from contextlib import ExitStack

    dt = mybir.dt
    f32, bf16 = dt.float32, dt.bfloat16
    D, CH = _D, _CH
    KB = C // 128          # contraction blocks for the projections
    NCH = T // CH          # 512-token chunks per batch (also q-chunks)
    NTT = T // 128         # 128-token tiles per batch (also k-tiles)
    BT = B * T
    F = HPC * D            # per-core head feature width
    AF = mybir.ActivationFunctionType

    nc = bacc.Bacc(
        "TRN2", target_bir_lowering=False, debug=False, num_devices=n_cores
    )

    xt = nc.dram_tensor("xt", [C, BT], bf16, kind="ExternalInput").ap()
    wq = nc.dram_tensor("wq", [C, F], bf16, kind="ExternalInput").ap()
    wk = nc.dram_tensor("wk", [C, F], bf16, kind="ExternalInput").ap()
    wv = nc.dram_tensor("wv", [C, F], bf16, kind="ExternalInput").ap()
    wo = nc.dram_tensor("wo", [F, COUT], bf16, kind="ExternalInput").ap()
    ccq = nc.dram_tensor("ccq", [128, T], bf16, kind="ExternalInput").ap()
    ssq = nc.dram_tensor("ssq", [128, T], bf16, kind="ExternalInput").ap()
    cck = nc.dram_tensor("cck", [128, T], bf16, kind="ExternalInput").ap()
    ssk = nc.dram_tensor("ssk", [128, T], bf16, kind="ExternalInput").ap()
    triu = nc.dram_tensor("triu", [128, 128], bf16, kind="ExternalInput").ap()
    onesb = nc.dram_tensor("onesb", [128, 1], bf16, kind="ExternalInput").ap()
    part = nc.dram_tensor("part", [BT, COUT], bf16, kind="ExternalOutput").ap()

    with ExitStack() as ctx:
        tc = ctx.enter_context(tile.TileContext(nc))
        const_pool = ctx.enter_context(tc.tile_pool(name="const", bufs=1))
        xc_pool = ctx.enter_context(tc.tile_pool(name="xc", bufs=2))
        rot_pool = ctx.enter_context(tc.tile_pool(name="rot", bufs=4 * HPC))
        swap_pool = ctx.enter_context(tc.tile_pool(name="swap", bufs=3))
        a_pool = ctx.enter_context(tc.tile_pool(name="ropeA", bufs=3))
        v_pool = ctx.enter_context(tc.tile_pool(name="vsb", bufs=2))
        u_pool = ctx.enter_context(tc.tile_pool(name="u", bufs=6))
        r_pool = ctx.enter_context(tc.tile_pool(name="recip", bufs=2))
        rb_pool = ctx.enter_context(tc.tile_pool(name="rbcast", bufs=2))
        y_pool = ctx.enter_context(tc.tile_pool(name="y", bufs=2 * HPC))
        o_pool = ctx.enter_context(tc.tile_pool(name="osb", bufs=3))
        ps_qk = ctx.enter_context(tc.tile_pool(name="psqk", bufs=2, space="PSUM"))
        ps_v = ctx.enter_context(tc.tile_pool(name="psv", bufs=1, space="PSUM"))
        ps_S = ctx.enter_context(tc.tile_pool(name="psS", bufs=2, space="PSUM"))
        ps_y = ctx.enter_context(tc.tile_pool(name="psy", bufs=2, space="PSUM"))
        ps_d = ctx.enter_context(tc.tile_pool(name="psd", bufs=1, space="PSUM"))
        ps_R = ps_qk  # broadcast tiles share the projection PSUM slots

        # ---- persistent constants ----
        # load order matters for startup latency: q/k weights + rope tables
        # feed the first matmuls; wv is needed a bit later, wo only at the
        # first output projection.
        wq_sb = const_pool.tile([128, KB * F], bf16, tag="wq")
        wk_sb = const_pool.tile([128, KB * F], bf16, tag="wk")
        for wsb, wdr in ((wq_sb, wq), (wk_sb, wk)):
            nc.sync.dma_start(
                wsb[:].rearrange("p (kb f) -> p kb f", kb=KB),
                wdr.rearrange("(kb p) f -> p kb f", p=128),
            )
        xc0 = xc_pool.tile([128, KB * CH], bf16, tag="xc", name="xc0")
        nc.sync.dma_start(
            xc0[:].rearrange("p (kb t) -> p kb t", kb=KB),
            xt.rearrange("(kb p) t -> p kb t", p=128)[:, :, 0:CH],
        )
        ccq_sb = const_pool.tile([128, T], bf16, tag="ccq")
        ssq_sb = const_pool.tile([128, T], bf16, tag="ssq")
        cck_sb = const_pool.tile([128, T], bf16, tag="cck")
        ssk_sb = const_pool.tile([128, T], bf16, tag="ssk")
        for csb, cdr in ((ccq_sb, ccq), (ssq_sb, ssq), (cck_sb, cck), (ssk_sb, ssk)):
            nc.sync.dma_start(csb[:], cdr)
        wv_sb = const_pool.tile([128, KB * F], bf16, tag="wv")
        nc.sync.dma_start(
            wv_sb[:].rearrange("p (kb f) -> p kb f", kb=KB),
            wv.rearrange("(kb p) f -> p kb f", p=128),
        )
        triu_sb = const_pool.tile([128, 128], bf16, tag="triu")
        nc.sync.dma_start(triu_sb[:], triu)
        onesb_sb = const_pool.tile([128, 1], bf16, tag="onesb")
        nc.sync.dma_start(onesb_sb[:], onesb)
        wo_sb = const_pool.tile([128, HPC * COUT], bf16, tag="wo")
        nc.sync.dma_start(
            wo_sb[:].rearrange("p (h n) -> p h n", h=HPC),
            wo.rearrange("(h p) n -> p h n", p=128),
        )

        # ---- PE warm-up: keep the HAM clock gate busy while the initial
        # DMA wave streams in, so the first real matmuls run at 2.4 GHz ----
        junk = const_pool.tile([128, 128], bf16, tag="warmjunk")
        nc.vector.memset(junk[:], 0)
        psw = ps_S.tile([128, 128], f32, tag="psS", name="pswarm")
        for _ in range(260):
            nc.tensor.matmul(
                psw[:], junk[:], junk[:], start=True, stop=True,
                skip_group_check=True,
            )

        # Deferred out-projection units (one q-chunk of token tiles each).
        # Emitted with a one-chunk lag so the softmax-normalization chain of
        # chunk j completes while PE runs chunk j+1's attention matmuls —
        # emitting immediately would park that latency on PE's in-order path.
        pending = []

        def emit_outproj():
            ysb_, b_, j_ = pending.pop(0)
            for m in range(j_ * (CH // 128), (j_ + 1) * (CH // 128)):
                osb = o_pool.tile([128, COUT], bf16, tag="osb", name="osb")
                for nch in range(COUT // CH):
                    pso = ps_qk.tile([128, CH], f32, tag="psqk", name="pso")
                    for h in range(HPC):
                        nc.tensor.matmul(
                            pso[:],
                            ysb_[h][:, m * 128 : (m + 1) * 128],
                            wo_sb[
                                :,
                                h * COUT + nch * CH : h * COUT + (nch + 1) * CH,
                            ],
                            start=(h == 0),
                            stop=(h == HPC - 1),
                            skip_group_check=True,
                        )
                    if nch % 2 == 0:
                        nc.scalar.copy(osb[:, nch * CH : (nch + 1) * CH], pso[:])
                    else:
                        nc.vector.tensor_copy(
                            osb[:, nch * CH : (nch + 1) * CH], pso[:]
                        )
                nc.sync.dma_start(
                    part[b_ * T + m * 128 : b_ * T + (m + 1) * 128, :], osb[:]
                )

        for b in range(B):
            # ---- qkv projection + RoPE for this batch ----
            qrot = [rot_pool.tile([128, T], bf16, tag="rot", name=f"qrot{h}") for h in range(HPC)]
            krot = [rot_pool.tile([128, T], bf16, tag="rot", name=f"krot{h}") for h in range(HPC)]
            vsb = v_pool.tile([128, NTT * F], bf16, tag="v")
            for c in range(NCH):
                if b == 0 and c == 0:
                    xc = xc0
                else:
                    xc = xc_pool.tile([128, KB * CH], bf16, tag="xc")
                    nc.sync.dma_start(
                        xc[:].rearrange("p (kb t) -> p kb t", kb=KB),
                        xt.rearrange("(kb p) t -> p kb t", p=128)[
                            :, :, b * T + c * CH : b * T + (c + 1) * CH
                        ],
                    )
                for h in range(HPC):
                    for wsb, ccs, sss, dst in (
                        (wq_sb, ccq_sb, ssq_sb, qrot[h]),
                        (wk_sb, cck_sb, ssk_sb, krot[h]),
                    ):
                        ps = ps_qk.tile([128, CH], f32, tag="psqk")
                        for kb in range(KB):
                            nc.tensor.matmul(
                                ps[:],
                                wsb[:, kb * F + h * D : kb * F + (h + 1) * D],
                                xc[:, kb * CH : (kb + 1) * CH],
                                start=(kb == 0),
                                stop=(kb == KB - 1),
                            )
                        # RoPE: rot = ps * cc + halfswap(ps) * ss
                        sw = swap_pool.tile([128, CH], bf16, tag="swap")
                        nc.vector.tensor_copy(sw[0:64, :], ps[64:128, :])
                        nc.vector.tensor_copy(sw[64:128, :], ps[0:64, :])
                        A = a_pool.tile([128, CH], f32, tag="ropeA")
                        nc.vector.tensor_mul(
                            A[:], ps[:], ccs[:, c * CH : (c + 1) * CH]
                        )
                        Bt = a_pool.tile([128, CH], bf16, tag="ropeB")
                        nc.vector.tensor_mul(
                            Bt[:], sw[:], sss[:, c * CH : (c + 1) * CH]
                        )
                        nc.vector.tensor_add(
                            dst[:, c * CH : (c + 1) * CH], A[:], Bt[:]
                        )
                # V in [token part, feature free] layout: x-blocks stationary
                for tm in range(CH // 128):
                    psv = ps_v.tile([128, F], f32, tag="psv")
                    for kb in range(KB):
                        nc.tensor.matmul(
                            psv[:],
                            xc[:, kb * CH + tm * 128 : kb * CH + tm * 128 + 128],
                            wv_sb[:, kb * F : (kb + 1) * F],
                            start=(kb == 0),
                            stop=(kb == KB - 1),
                        )
                    tt = c * (CH // 128) + tm
                    nc.scalar.copy(vsb[:, tt * F : (tt + 1) * F], psv[:])

            # ---- attention per head, out-projection interleaved per q-chunk ----
            ysb = [y_pool.tile([128, T], bf16, tag="y", name=f"ysb{h}") for h in range(HPC)]
            for j in range(NCH):
                for h in range(HPC):
                    psy = ps_y.tile([128, CH], f32, tag="psy")
                    psd = ps_d.tile([1, CH], f32, tag="psd")
                    nkb = (CH // 128) * (j + 1)  # causal: k-tiles <= q-chunk end
                    for kb in range(nkb):
                        c0 = max(0, kb * 128 - j * CH)
                        psS = ps_S.tile([128, CH], f32, tag="psS")
                        nc.tensor.matmul(
                            psS[:, c0:CH],
                            krot[h][:, kb * 128 : (kb + 1) * 128],
                            qrot[h][:, j * CH + c0 : (j + 1) * CH],
                            start=True,
                            stop=True,
                        )
                        U = u_pool.tile([128, CH], bf16, tag="u")
                        nc.scalar.activation(U[:, c0:CH], psS[:, c0:CH], AF.Exp)
                        if kb * 128 >= j * CH:
                            # diagonal 128x128 block: zero out k > q after exp
                            nc.vector.tensor_mul(
                                U[:, c0 : c0 + 128],
                                U[:, c0 : c0 + 128],
                                triu_sb[:],
                            )
                        nc.tensor.matmul(
                            psy[:, c0:CH],
                            vsb[:, kb * F + h * D : kb * F + (h + 1) * D],
                            U[:, c0:CH],
                            start=(kb == 0),
                            stop=(kb == nkb - 1),
                            skip_group_check=True,
                        )
                        nc.tensor.matmul(
                            psd[:, c0:CH],
                            onesb_sb[:],
                            U[:, c0:CH],
                            start=(kb == 0),
                            stop=(kb == nkb - 1),
                            skip_group_check=True,
                        )
                    rr = r_pool.tile([1, CH], f32, tag="recip")
                    nc.vector.reciprocal_approx_fast(rr[:], psd[:])
                    Rsb = rb_pool.tile([128, CH], f32, tag="rbcast")
                    nc.gpsimd.partition_broadcast(Rsb[:], rr[:])
                    nc.vector.tensor_mul(
                        ysb[h][:, j * CH : (j + 1) * CH], psy[:], Rsb[:]
                    )

                # defer this chunk's out-projection by one chunk
                pending.append((ysb, b, j))
                if len(pending) > 1:
                    emit_outproj()

        while pending:
            emit_outproj()

    nc.compile()
    return nc


def make_host_inputs(x, cos, sin, Wqkv, Wout, H, n_cores):
    """Shard + precompute the per-core device input maps (numpy, host side)."""
    bf16 = ml_dtypes.bfloat16
    B, T, C = x.shape
    D = C // H
    HPC = H // n_cores
    COUT = Wout.shape[1]

    xt = np.ascontiguousarray(x.reshape(B * T, C).T).astype(bf16)

    # deinterleave permutation within each head: [0,2,4,...,1,3,5,...]
    perm = np.concatenate([np.arange(0, D, 2), np.arange(1, D, 2)])
    Wq = Wqkv[:, 0:C].reshape(C, H, D)[:, :, perm]
    Wk = Wqkv[:, C : 2 * C].reshape(C, H, D)[:, :, perm]
    Wv = Wqkv[:, 2 * C : 3 * C].reshape(C, H, D)

    cosT = cos.T  # [D/2, T]
    CC = np.concatenate([cosT, cosT], axis=0)  # [D, T]
    SS = np.concatenate([-sin.T, sin.T], axis=0)
    scale = 1.0 / math.sqrt(D)
    ccq = (CC * scale).astype(bf16)
    ssq = (SS * scale).astype(bf16)
    cck = CC.astype(bf16)
    ssk = SS.astype(bf16)

    tri = np.triu(np.ones((128, 128), dtype=np.float32)).astype(bf16)
    onesb = np.ones((128, 1), dtype=np.float32).astype(bf16)

    in_maps = []
    for core in range(n_cores):
        hs = slice(core * HPC, (core + 1) * HPC)
        in_maps.append(
            {
                "xt": xt,
                "wq": np.ascontiguousarray(
                    Wq[:, hs, :].reshape(C, HPC * D)
                ).astype(bf16),
                "wk": np.ascontiguousarray(
                    Wk[:, hs, :].reshape(C, HPC * D)
                ).astype(bf16),
                "wv": np.ascontiguousarray(
                    Wv[:, hs, :].reshape(C, HPC * D)
                ).astype(bf16),
                "wo": np.ascontiguousarray(
                    Wout[core * HPC * D : (core + 1) * HPC * D, :]
                ).astype(bf16),
                "ccq": ccq,
                "ssq": ssq,
                "cck": cck,
                "ssk": ssk,
                "triu": tri,
                "onesb": onesb,
            }
        )
    return in_maps


_PROGRAM_CACHE = {}


def kernel(x, cos, sin, Wqkv, Wout):
    global LAST_RESULT
    from concourse.bass_utils import run_bass_kernel_spmd

    x = np.asarray(x, dtype=np.float32)
    cos = np.asarray(cos, dtype=np.float32)
    sin = np.asarray(sin, dtype=np.float32)
    Wqkv = np.asarray(Wqkv, dtype=np.float32)
    Wout = np.asarray(Wout, dtype=np.float32)

    B, T, C = x.shape
    H = _H
    COUT = Wout.shape[1]
    n_cores = 8
    HPC = H // n_cores

    key = (B, T, C, COUT, HPC, n_cores)
    if key not in _PROGRAM_CACHE:
        _PROGRAM_CACHE[key] = build_program(B, T, C, COUT, HPC, n_cores)
    nc = _PROGRAM_CACHE[key]

    in_maps = make_host_inputs(x, cos, sin, Wqkv, Wout, H, n_cores)
    res = run_bass_kernel_spmd(
        nc, in_maps, core_ids=list(range(n_cores)), trace=TRACE
    )
    LAST_RESULT = res

    out = np.zeros((B * T, COUT), dtype=np.float32)
    for r in res.results:
        out += np.asarray(r["part"], dtype=np.float32)
    return out.reshape(B, T, COUT)
